# revision 18
# baseline (speedup 1.0000x reference)
"""DropPart masking kernel for Trainium2 (8 NeuronCores, data-parallel over batch).

Problem: x (64, 256, 96, 32) f32. Per sample n and channel-group g (8 groups x
32 channels), a keypoint defines a keep-box; if roll[n,g] < 0.5 the group's
channels are zeroed outside the box (box <= 16x16 in the 96x32 image), else the
group passes through unchanged.

The op is pure data movement: every (n,g) "slot" ([32ch, 96, 32] = 384KB) is
either identity or (zeros outside an axis-aligned box, x inside). This kernel
runs IN-PLACE on the device: the full x tensor is donated as the buffer backing
the NEFF's output (XLA donation aliases the param to the custom-call result, the
same mechanism bass2jax.run_bass_via_pjrt uses for its zero-filled outputs), so
identity slots need zero HBM traffic, and each masked slot costs one window
read (128KB) + one full-slab zero write (384KB) + one window write (128KB)
instead of the 768KB full read+write. At ~50% masked slots this is ~21MB/core
round-trip vs 50MB for the out-of-place kernel -- and the f32 output is
bit-exact.

Per-core slot schedules are data (which slots are masked, where the box rows
sit), so a single SPMD program handles them via dynamic-offset DMAs: an i32
metadata input provides (slot, chunk, flag) per work position; a cond-predicated
DMA whose offsets come from engine registers executes the transfer, and padding
positions are skipped via cond=0 (skip_entire_dma). The 1024-col box window
([32, 2, 512] at a 512-aligned dynamic offset) is a single DMA via a manually
widened access pattern. Mask values are applied with the eyes-matmul expansion
(PE) + one tensor_mul per 512-chunk (DVE); zeros outside the window come from a
memset SBUF tile. Each slot's load -> zero-slab store -> window store are issued
on one HWDGE queue in order (slots alternate SP/ACT queues), which both orders
the overlapping slab transfers and splits the store bandwidth across rings.

The program is input-independent (one compile per work-list capacity W, cached);
the For_i(nreps) wrapper supports in-NEFF repetition for timing. The body is
idempotent (mask o mask = mask), so repeated in-place application is safe.
"""

import numpy as np
import ml_dtypes

import bass_rust
import concourse.bass as bass
import concourse.bacc as bacc
import concourse.tile as tile
from concourse import mybir

N, C, H, W = 64, 256, 96, 32
GROUPS = 8
P_DROP = 0.5
HW = H * W              # 3072
CHS = C // GROUPS       # 32
N_CORES = 8
NPC = N // N_CORES      # samples per core = 8
SLOTS = NPC * GROUPS    # (sample, group) slots per core = 64
NCHUNK = 6              # 512-element chunks per image (96*32 / 512)

_F32 = mybir.dt.float32
_BF16 = mybir.dt.bfloat16
_I32 = mybir.dt.int32

_SP = mybir.EngineType.SP
_ACT = mybir.EngineType.Activation
_POOL = mybir.EngineType.Pool


SLAB = CHS * HW  # elements per slot slab = 98304


def _win_ap(o4, s, c, p):
    """[32, 2, 512] window AP at slab s, chunks [c, c+2): widen the
    [32, 512] single-chunk slice with an extra (stride=512, size=2) dim.

    dep_tracking_offset is pinned to a static per-work-position fake slab so
    the Tile dependency tracker sees disjoint regions per slot (the dynamic
    offsets would otherwise serialize every DMA against the whole tensor).
    Within a position, the window load / zero store / window store share the
    fake region, preserving their required ordering; distinct positions touch
    provably disjoint real slabs, so dropping those edges is sound."""
    a = o4[s, :, c, :].copy()
    a.ap = bass_rust.VecI64Pair([(HW, CHS), (512, 2), (1, 512)])
    a.dep_tracking_offset = p * SLAB
    return a


def _slab_ap(o4, s, p):
    """[32, 6, 512] full-slab AP at slab s, same fake dep region as _win_ap."""
    a = o4[s].copy()
    a.dep_tracking_offset = p * SLAB
    return a


def _unit_ap(o4, s, p):
    """[64, 6, 512] two-consecutive-slab AP starting at slab s (s <= 62).
    dep_tracking_offset = static fake region per work position (see _win_ap)."""
    a = o4[s].copy()
    a.ap = bass_rust.VecI64Pair([(HW, 2 * CHS), (512, NCHUNK), (1, 512)])
    a.dep_tracking_offset = p * 2 * SLAB
    return a


def _build_module(w_items: int, xbufs: int = 6, mulw: int = 1024):
    """One SPMD module processing w_items*4 slot positions per iteration.

    Per slot position p (metadata: slot s, active flag f), all offsets coming
    from engine registers so one compiled program serves every core's
    schedule:
      L: DMA slab s ([32, 6, 512] f32, 384KB) into the item's [128, 3072]
         SBUF tile (cond=f; inactive positions skip via skip_entire_dma).
      mask multiply: PE expands the 4 packed mask rows (one per 32-channel
         lane) to 128 partitions via the eyes matmul; DVE multiplies the
         whole tile. Masked-slot masks are 1 inside the keep-box and 0
         outside, so the product is exactly the reference's x*mask.
      S: DMA the slab back (cond=f). S chains after L through the SBUF
         dependency (mul), so queues are free; cross-iteration window races
         in the timing loop are benign because masking is idempotent.
    L and S rotate across the SP / ACT HWDGE queues and the gpsimd SWDGE
    queue to spread descriptor-processing overhead and store bandwidth.
    """
    nunits = w_items * 2
    nc = bacc.Bacc("TRN2", target_bir_lowering=False, debug=False)

    o4 = nc.dram_tensor("out", [SLOTS, CHS, NCHUNK, 512], _F32, kind="ExternalOutput").ap()
    meta_d = nc.dram_tensor("meta", [1, nunits * 4], _I32, kind="ExternalInput").ap()
    mpk_d = nc.dram_tensor("mpk", [4, w_items * HW], _BF16, kind="ExternalInput").ap()
    eyes_d = nc.dram_tensor("eyes", [4, 128], _BF16, kind="ExternalInput").ap()
    reps_d = nc.dram_tensor("nreps", [1, 1], _I32, kind="ExternalInput").ap()

    queues = [(nc.sync, _SP), (nc.scalar, _ACT), (nc.gpsimd, _POOL)]
    nbank = mulw // 512

    with tile.TileContext(nc) as tc:
        with (
            tc.tile_pool(name="consts", bufs=1) as consts,
            tc.tile_pool(name="xpool", bufs=xbufs) as xpool,
            tc.tile_pool(name="psum", bufs=8 // nbank, space="PSUM") as psum,
        ):
            mt = consts.tile([1, nunits * 4], _I32)
            nc.sync.dma_start(mt[:], meta_d[:])
            mpk = consts.tile([4, w_items * HW], _BF16)
            nc.sync.dma_start(mpk[:], mpk_d[:])
            eyes = consts.tile([4, 128], _BF16)
            nc.sync.dma_start(eyes[:], eyes_d[:])
            rtile = consts.tile([1, 1], _I32)
            nc.sync.dma_start(rtile[:], reps_d[:])

            with tc.For_i(0, nc.values_load(rtile[0:1, 0:1]), 1):
                for it in range(w_items):
                    xt = xpool.tile([128, HW], _F32)
                    regs = []
                    for u in range(2):  # 2 two-slot units per item
                        p = it * 2 + u
                        qL, engL = queues[p % 3]
                        qS, engS = queues[(p + 1) % 3]
                        engs = [engL] if engS == engL else [engL, engS]
                        s = nc.values_load(mt[0:1, 4 * p : 4 * p + 1], engines=engs,
                                           min_val=0, max_val=SLOTS - 2,
                                           skip_runtime_bounds_check=True)
                        f = nc.values_load(mt[0:1, 4 * p + 2 : 4 * p + 3], engines=engs,
                                           min_val=0, max_val=1,
                                           skip_runtime_bounds_check=True)
                        regs.append((qS, s, f))
                        xk3 = xt[64 * u : 64 * u + 64, :].rearrange("p (a b) -> p a b", b=512)
                        qL.dma_start(xk3, _unit_ap(o4, s, p), cond=f, cond_hint=True)
                    for j in range(HW // mulw):
                        pt = psum.tile([128, mulw], _F32)
                        for b in range(nbank):
                            col = it * HW + j * mulw + b * 512
                            nc.tensor.matmul(pt[:, 512 * b : 512 * b + 512], eyes[:],
                                             mpk[:, col : col + 512],
                                             start=True, stop=True)
                        nc.vector.tensor_mul(xt[:, j * mulw : (j + 1) * mulw],
                                             xt[:, j * mulw : (j + 1) * mulw], pt[:])
                    for u in range(2):
                        p = it * 2 + u
                        qS, s, f = regs[u]
                        xk3 = xt[64 * u : 64 * u + 64, :].rearrange("p (a b) -> p a b", b=512)
                        qS.dma_start(_unit_ap(o4, s, p), xk3, cond=f, cond_hint=True)

    nc.compile()
    return nc


_MODULES: dict = {}


def _get_module(w_items: int):
    if w_items not in _MODULES:
        _MODULES[w_items] = _build_module(w_items)
    return _MODULES[w_items]


def _host_masks(key_pts: np.ndarray, roll: np.ndarray) -> np.ndarray:
    """Per-(n,g) masks [N, GROUPS, H*W] in {0,1} f32, math exactly as reference."""
    s = int(0.25 * W)
    kx = (key_pts[:, :GROUPS, 0] * np.float32(W)).astype(np.float32)
    ky = (key_pts[:, :GROUPS, 1] * np.float32(H)).astype(np.float32)
    cond = (roll[:, :GROUPS] < np.float32(P_DROP)) & (kx >= 0) & (ky >= 0)

    bx = np.floor(np.maximum(kx - s, np.float32(0.0)))
    ex = np.floor(np.minimum(kx + s, np.float32(W)))
    by = np.floor(np.maximum(ky - s, np.float32(0.0)))
    ey = np.floor(np.minimum(ky + s, np.float32(H)))

    xs = np.arange(W, dtype=np.float32)
    ys = np.arange(H, dtype=np.float32)
    inx = (xs[None, None, :] >= bx[:, :, None]) & (xs[None, None, :] < ex[:, :, None])
    iny = (ys[None, None, :] >= by[:, :, None]) & (ys[None, None, :] < ey[:, :, None])
    box = iny[:, :, :, None] & inx[:, :, None, :]  # [N, G, H, W] bool

    mask = np.where(cond[:, :, None, None], box, True)
    return mask.reshape(N, GROUPS, HW).astype(np.float32)


def build_schedule(key_pts: np.ndarray, roll: np.ndarray):
    """Host schedule: per-core packed work lists from the mask table.

    Returns (w_items, metas [8][1, nslots*4] i32, mpks [8][nslots, 1024] bf16).
    A slot is active iff its mask differs from all-ones; its window chunk c is
    chosen from the mask's nonzero rows so that chunks [c, c+2) cover them.
    """
    masks = _host_masks(key_pts, roll)  # [N, G, HW] f32 0/1
    m_core = masks.reshape(N_CORES, SLOTS, H, W)
    # Cover each core's masked-slot set with disjoint 2-slot intervals
    # [s, s+2). An interval may include an identity neighbor: its mask lane is
    # all-ones, so the rewrite is value-preserving. Fewer, bigger DMAs.
    work = []  # per core: list of unit start slots
    for cidx in range(N_CORES):
        masked = [sl for sl in range(SLOTS) if m_core[cidx, sl].min() < 1.0]
        units, nxt = [], 0
        for sl in masked:
            if sl < nxt:
                continue
            start = min(sl, SLOTS - 2)
            units.append(start)
            nxt = start + 2
        work.append(units)

    u_max = max(len(lst) for lst in work)
    w_items = min(32, max(1, -(-u_max // 2)))  # 2 units per item
    nunits = w_items * 2

    metas, mpks = [], []
    flat = masks.reshape(N_CORES, SLOTS, HW)
    for cidx in range(N_CORES):
        meta = np.zeros((1, nunits * 4), dtype=np.int32)
        mpk = np.zeros((4, w_items * HW), dtype=np.float32)  # [lane k, item i cols]
        for p, start in enumerate(work[cidx]):
            meta[0, 4 * p : 4 * p + 3] = (start, 0, 1)
            it, u = divmod(p, 2)
            for j in range(2):
                mpk[2 * u + j, it * HW : (it + 1) * HW] = flat[cidx, start + j]
        metas.append(meta)
        mpks.append(mpk.astype(ml_dtypes.bfloat16))
    return w_items, metas, mpks


def _eyes4() -> np.ndarray:
    e = np.zeros((4, 128), dtype=np.float32)
    for k in range(4):
        e[k, 32 * k : 32 * k + 32] = 1.0
    return e.astype(ml_dtypes.bfloat16)


def make_runner(nc):
    """jit'd shard_map runner over 8 cores with the 'out' buffer donated.

    Returns fn(meta_g, mpk_g, eyes_g, nreps_g, out_g) -> out_g ([512,32,6,512]
    f32 jax array). out_g is consumed (donated); chain calls by passing the
    previous result.
    """
    import jax
    from jax.sharding import Mesh, PartitionSpec
    from jax.experimental.shard_map import shard_map
    from concourse.bass2jax import (
        _bass_exec_p,
        install_neuronx_cc_hook,
        partition_id_tensor,
    )

    install_neuronx_cc_hook()
    partition_name = nc.partition_id_tensor.name if nc.partition_id_tensor else None

    in_names, out_names, out_avals = [], [], []
    for alloc in nc.m.functions[0].allocations:
        if not isinstance(alloc, mybir.MemoryLocationSet):
            continue
        name = alloc.memorylocations[0].name
        if alloc.kind == "ExternalInput":
            if name != partition_name:
                in_names.append(name)
        elif alloc.kind == "ExternalOutput":
            out_names.append(name)
            out_avals.append(jax.core.ShapedArray(tuple(alloc.tensor_shape),
                                                  mybir.dt.np(alloc.dtype)))
    assert out_names == ["out"]
    order = ["meta", "mpk", "eyes", "nreps"]
    assert sorted(in_names) == sorted(order), in_names
    perm = [order.index(n) for n in in_names]
    all_names = tuple(in_names) + ("out",)
    if partition_name is not None:
        all_names = all_names + (partition_name,)

    def _body(*args):
        operands = [args[perm[i]] for i in range(4)] + [args[4]]
        if partition_name is not None:
            operands.append(partition_id_tensor())
        (res,) = _bass_exec_p.bind(
            *operands,
            out_avals=tuple(out_avals),
            in_names=all_names,
            out_names=("out",),
            lowering_input_output_aliases=(),
            sim_require_finite=False, sim_require_nnan=False, nc=nc)
        return (res,)

    mesh = Mesh(np.asarray(jax.devices()[:N_CORES]), ("core",))
    specs = (PartitionSpec("core"),) * 5
    fn = jax.jit(
        shard_map(_body, mesh=mesh, in_specs=specs,
                  out_specs=(PartitionSpec("core"),), check_rep=False),
        donate_argnums=(4,), keep_unused=True)
    return fn, mesh


def kernel(x: np.ndarray, key_pts: np.ndarray, roll: np.ndarray, **_kw) -> np.ndarray:
    import jax
    from jax.sharding import NamedSharding, PartitionSpec

    x = np.ascontiguousarray(np.asarray(x, dtype=np.float32))
    key_pts = np.asarray(key_pts, dtype=np.float32)
    roll = np.asarray(roll, dtype=np.float32)

    w_items, metas, mpks = build_schedule(key_pts, roll)
    nc = _get_module(w_items)
    fn, mesh = make_runner(nc)
    sharding = NamedSharding(mesh, PartitionSpec("core"))

    meta_g = jax.device_put(np.concatenate(metas, axis=0), sharding)
    mpk_g = jax.device_put(np.concatenate(mpks, axis=0), sharding)
    eyes_g = jax.device_put(np.concatenate([_eyes4()] * N_CORES, axis=0), sharding)
    reps_g = jax.device_put(np.ones((N_CORES, 1), np.int32), sharding)
    out_g = jax.device_put(x.reshape(N * GROUPS, CHS, NCHUNK, 512), sharding)

    res = fn(meta_g, mpk_g, eyes_g, reps_g, out_g)[0]
    return np.asarray(res).reshape(N, C, H, W)


# revision 19
# speedup vs baseline: 1.0587x; 1.0587x over previous
"""DropPart masking kernel for Trainium2 (8 NeuronCores, data-parallel over batch).

Problem: x (64, 256, 96, 32) f32. Per sample n and channel-group g (8 groups x
32 channels), a keypoint defines a keep-box; if roll[n,g] < 0.5 the group's
channels are zeroed outside the box (box <= 16x16 in the 96x32 image), else the
group passes through unchanged.

The op is pure data movement plus a 0/1 mask multiply: every (n,g) "slot"
([32ch, 96, 32] = 384KB) is either identity or x*boxmask. This kernel runs
IN-PLACE on the device: the full x tensor is donated as the buffer backing the
NEFF's output (XLA donation aliases the param to the custom-call result -- the
same mechanism bass2jax.run_bass_via_pjrt relies on for its zero-filled output
buffers), so identity slots need ZERO HBM traffic and only masked slots are
touched: one 384KB slab read + one 384KB slab write each. At ~50% masked slots
that is ~32MB/core round-trip vs 50MB for the out-of-place streaming kernel,
and the f32 output is bit-exact (rel err 0.0).

Which slots are masked is per-core data, so a single SPMD program executes a
metadata-driven schedule: per work position an i32 input supplies (slot s,
flag f); the slab load/store DMAs take their DRAM offsets from engine
registers (values_load) and padding positions are skipped via cond=f
(skip_entire_dma). dep_tracking_offset on each dynamic AP is pinned to a
static per-position fake region so the Tile scheduler sees disjoint slots
instead of serializing every DMA against the whole output tensor. The mask
multiply uses the eyes-matmul expansion (PE, [4,128] one-hot weights applied
to 4 packed bf16 mask rows) + DVE tensor_mul; masked-slot masks are 1 inside
the keep-box and 0 outside, so the product equals the reference's x*mask
bit-for-bit, and all-ones lanes pass identity data through unchanged. Slab
DMAs rotate across three queues (SP + ACT HWDGE, gpsimd SWDGE) -- the
measured bottleneck is per-dynamic-DMA queue overhead (~2-3us each), not
bandwidth, so the third queue and the one-load-one-store-per-slot structure
are what set the 129us/iter (vs 154us baseline full-stream) timing.

The program is input-independent (one compile per work-list capacity, cached);
the For_i(nreps) wrapper supports in-NEFF repetition for timing. The body is
idempotent (mask o mask = mask), so repeated in-place application is safe.
"""

import numpy as np
import ml_dtypes

import bass_rust
import concourse.bass as bass
import concourse.bacc as bacc
import concourse.tile as tile
from concourse import mybir

N, C, H, W = 64, 256, 96, 32
GROUPS = 8
P_DROP = 0.5
HW = H * W              # 3072
CHS = C // GROUPS       # 32
N_CORES = 8
NPC = N // N_CORES      # samples per core = 8
SLOTS = NPC * GROUPS    # (sample, group) slots per core = 64
NCHUNK = 6              # 512-element chunks per image (96*32 / 512)

_F32 = mybir.dt.float32
_BF16 = mybir.dt.bfloat16
_I32 = mybir.dt.int32

_SP = mybir.EngineType.SP
_ACT = mybir.EngineType.Activation
_POOL = mybir.EngineType.Pool


SLAB = CHS * HW  # elements per slot slab = 98304


def _win_ap(o4, s, c, p):
    """[32, 2, 512] window AP at slab s, chunks [c, c+2): widen the
    [32, 512] single-chunk slice with an extra (stride=512, size=2) dim.

    dep_tracking_offset is pinned to a static per-work-position fake slab so
    the Tile dependency tracker sees disjoint regions per slot (the dynamic
    offsets would otherwise serialize every DMA against the whole tensor).
    Within a position, the window load / zero store / window store share the
    fake region, preserving their required ordering; distinct positions touch
    provably disjoint real slabs, so dropping those edges is sound."""
    a = o4[s, :, c, :].copy()
    a.ap = bass_rust.VecI64Pair([(HW, CHS), (512, 2), (1, 512)])
    a.dep_tracking_offset = p * SLAB
    return a


def _slab_ap(o4, s, p):
    """[32, 6, 512] full-slab AP at slab s, same fake dep region as _win_ap."""
    a = o4[s].copy()
    a.dep_tracking_offset = p * SLAB
    return a


def _build_module(w_items: int, xbufs: int = 6, mulw: int = 1024):
    """One SPMD module processing w_items*4 slot positions per iteration.

    Per slot position p (metadata: slot s, active flag f), all offsets coming
    from engine registers so one compiled program serves every core's
    schedule:
      L: DMA slab s ([32, 6, 512] f32, 384KB) into the item's [128, 3072]
         SBUF tile (cond=f; inactive positions skip via skip_entire_dma).
      mask multiply: PE expands the 4 packed mask rows (one per 32-channel
         lane) to 128 partitions via the eyes matmul; DVE multiplies the
         whole tile. Masked-slot masks are 1 inside the keep-box and 0
         outside, so the product is exactly the reference's x*mask.
      S: DMA the slab back (cond=f). S chains after L through the SBUF
         dependency (mul), so queues are free; cross-iteration window races
         in the timing loop are benign because masking is idempotent.
    L and S rotate across the SP / ACT HWDGE queues and the gpsimd SWDGE
    queue to spread descriptor-processing overhead and store bandwidth.
    """
    nslots = w_items * 4
    nc = bacc.Bacc("TRN2", target_bir_lowering=False, debug=False)

    o4 = nc.dram_tensor("out", [SLOTS, CHS, NCHUNK, 512], _F32, kind="ExternalOutput").ap()
    meta_d = nc.dram_tensor("meta", [1, nslots * 4], _I32, kind="ExternalInput").ap()
    mpk_d = nc.dram_tensor("mpk", [4, w_items * HW], _BF16, kind="ExternalInput").ap()
    eyes_d = nc.dram_tensor("eyes", [4, 128], _BF16, kind="ExternalInput").ap()
    reps_d = nc.dram_tensor("nreps", [1, 1], _I32, kind="ExternalInput").ap()

    queues = [(nc.sync, _SP), (nc.scalar, _ACT), (nc.gpsimd, _POOL)]
    nbank = mulw // 512

    with tile.TileContext(nc) as tc:
        with (
            tc.tile_pool(name="consts", bufs=1) as consts,
            tc.tile_pool(name="xpool", bufs=xbufs) as xpool,
            tc.tile_pool(name="psum", bufs=8 // nbank, space="PSUM") as psum,
        ):
            mt = consts.tile([1, nslots * 4], _I32)
            nc.sync.dma_start(mt[:], meta_d[:])
            mpk = consts.tile([4, w_items * HW], _BF16)
            nc.sync.dma_start(mpk[:], mpk_d[:])
            eyes = consts.tile([4, 128], _BF16)
            nc.sync.dma_start(eyes[:], eyes_d[:])
            rtile = consts.tile([1, 1], _I32)
            nc.sync.dma_start(rtile[:], reps_d[:])

            with tc.For_i(0, nc.values_load(rtile[0:1, 0:1]), 1):
                for it in range(w_items):
                    xt = xpool.tile([128, HW], _F32)
                    regs = []
                    for k in range(4):
                        p = it * 4 + k
                        qL, engL = queues[p % 3]
                        qS, engS = queues[(p + 1) % 3]
                        engs = [engL] if engS == engL else [engL, engS]
                        s = nc.values_load(mt[0:1, 4 * p : 4 * p + 1], engines=engs,
                                           min_val=0, max_val=SLOTS - 1,
                                           skip_runtime_bounds_check=True)
                        f = nc.values_load(mt[0:1, 4 * p + 2 : 4 * p + 3], engines=engs,
                                           min_val=0, max_val=1,
                                           skip_runtime_bounds_check=True)
                        regs.append((qS, s, f))
                        xk3 = xt[32 * k : 32 * k + 32, :].rearrange("p (a b) -> p a b", b=512)
                        qL.dma_start(xk3, _slab_ap(o4, s, p), cond=f, cond_hint=True)
                    for j in range(HW // mulw):
                        pt = psum.tile([128, mulw], _F32)
                        for b in range(nbank):
                            col = it * HW + j * mulw + b * 512
                            nc.tensor.matmul(pt[:, 512 * b : 512 * b + 512], eyes[:],
                                             mpk[:, col : col + 512],
                                             start=True, stop=True)
                        nc.vector.tensor_mul(xt[:, j * mulw : (j + 1) * mulw],
                                             xt[:, j * mulw : (j + 1) * mulw], pt[:])
                    for k in range(4):
                        p = it * 4 + k
                        qS, s, f = regs[k]
                        xk3 = xt[32 * k : 32 * k + 32, :].rearrange("p (a b) -> p a b", b=512)
                        qS.dma_start(_slab_ap(o4, s, p), xk3, cond=f, cond_hint=True)

    nc.compile()
    return nc


_MODULES: dict = {}


def _get_module(w_items: int):
    if w_items not in _MODULES:
        _MODULES[w_items] = _build_module(w_items)
    return _MODULES[w_items]


def _host_masks(key_pts: np.ndarray, roll: np.ndarray) -> np.ndarray:
    """Per-(n,g) masks [N, GROUPS, H*W] in {0,1} f32, math exactly as reference."""
    s = int(0.25 * W)
    kx = (key_pts[:, :GROUPS, 0] * np.float32(W)).astype(np.float32)
    ky = (key_pts[:, :GROUPS, 1] * np.float32(H)).astype(np.float32)
    cond = (roll[:, :GROUPS] < np.float32(P_DROP)) & (kx >= 0) & (ky >= 0)

    bx = np.floor(np.maximum(kx - s, np.float32(0.0)))
    ex = np.floor(np.minimum(kx + s, np.float32(W)))
    by = np.floor(np.maximum(ky - s, np.float32(0.0)))
    ey = np.floor(np.minimum(ky + s, np.float32(H)))

    xs = np.arange(W, dtype=np.float32)
    ys = np.arange(H, dtype=np.float32)
    inx = (xs[None, None, :] >= bx[:, :, None]) & (xs[None, None, :] < ex[:, :, None])
    iny = (ys[None, None, :] >= by[:, :, None]) & (ys[None, None, :] < ey[:, :, None])
    box = iny[:, :, :, None] & inx[:, :, None, :]  # [N, G, H, W] bool

    mask = np.where(cond[:, :, None, None], box, True)
    return mask.reshape(N, GROUPS, HW).astype(np.float32)


def build_schedule(key_pts: np.ndarray, roll: np.ndarray):
    """Host schedule: per-core packed work lists from the mask table.

    Returns (w_items, metas [8][1, nslots*4] i32, mpks [8][nslots, 1024] bf16).
    A slot is active iff its mask differs from all-ones; its window chunk c is
    chosen from the mask's nonzero rows so that chunks [c, c+2) cover them.
    """
    masks = _host_masks(key_pts, roll)  # [N, G, HW] f32 0/1
    m_core = masks.reshape(N_CORES, SLOTS, H, W)
    work = []  # per core: list of (slot, chunk)
    for cidx in range(N_CORES):
        lst = []
        for sl in range(SLOTS):
            m = m_core[cidx, sl]
            if m.min() >= 1.0:
                continue  # identity slot
            rows = np.flatnonzero(m.any(axis=1))
            if rows.size == 0:
                ch = 0
            else:
                ch = min(int(rows[0]) // 16, NCHUNK - 2)
                assert int(rows[-1]) < 16 * ch + 32, (cidx, sl, rows[0], rows[-1])
            lst.append((sl, ch))
        work.append(lst)

    a_max = max(len(lst) for lst in work)
    w_items = min(16, max(1, -(-a_max // 4)))
    nslots = w_items * 4

    metas, mpks = [], []
    flat = masks.reshape(N_CORES, SLOTS, HW)
    for cidx in range(N_CORES):
        meta = np.zeros((1, nslots * 4), dtype=np.int32)
        mpk = np.zeros((4, w_items * HW), dtype=np.float32)  # [lane k, item i cols]
        for p, (sl, ch) in enumerate(work[cidx]):
            meta[0, 4 * p : 4 * p + 3] = (sl, ch, 1)
            it, k = divmod(p, 4)
            mpk[k, it * HW : (it + 1) * HW] = flat[cidx, sl]
        metas.append(meta)
        mpks.append(mpk.astype(ml_dtypes.bfloat16))
    return w_items, metas, mpks


def _eyes4() -> np.ndarray:
    e = np.zeros((4, 128), dtype=np.float32)
    for k in range(4):
        e[k, 32 * k : 32 * k + 32] = 1.0
    return e.astype(ml_dtypes.bfloat16)


def make_runner(nc):
    """jit'd shard_map runner over 8 cores with the 'out' buffer donated.

    Returns fn(meta_g, mpk_g, eyes_g, nreps_g, out_g) -> out_g ([512,32,6,512]
    f32 jax array). out_g is consumed (donated); chain calls by passing the
    previous result.
    """
    import jax
    from jax.sharding import Mesh, PartitionSpec
    from jax.experimental.shard_map import shard_map
    from concourse.bass2jax import (
        _bass_exec_p,
        install_neuronx_cc_hook,
        partition_id_tensor,
    )

    install_neuronx_cc_hook()
    partition_name = nc.partition_id_tensor.name if nc.partition_id_tensor else None

    in_names, out_names, out_avals = [], [], []
    for alloc in nc.m.functions[0].allocations:
        if not isinstance(alloc, mybir.MemoryLocationSet):
            continue
        name = alloc.memorylocations[0].name
        if alloc.kind == "ExternalInput":
            if name != partition_name:
                in_names.append(name)
        elif alloc.kind == "ExternalOutput":
            out_names.append(name)
            out_avals.append(jax.core.ShapedArray(tuple(alloc.tensor_shape),
                                                  mybir.dt.np(alloc.dtype)))
    assert out_names == ["out"]
    order = ["meta", "mpk", "eyes", "nreps"]
    assert sorted(in_names) == sorted(order), in_names
    perm = [order.index(n) for n in in_names]
    all_names = tuple(in_names) + ("out",)
    if partition_name is not None:
        all_names = all_names + (partition_name,)

    def _body(*args):
        operands = [args[perm[i]] for i in range(4)] + [args[4]]
        if partition_name is not None:
            operands.append(partition_id_tensor())
        (res,) = _bass_exec_p.bind(
            *operands,
            out_avals=tuple(out_avals),
            in_names=all_names,
            out_names=("out",),
            lowering_input_output_aliases=(),
            sim_require_finite=False, sim_require_nnan=False, nc=nc)
        return (res,)

    mesh = Mesh(np.asarray(jax.devices()[:N_CORES]), ("core",))
    specs = (PartitionSpec("core"),) * 5
    fn = jax.jit(
        shard_map(_body, mesh=mesh, in_specs=specs,
                  out_specs=(PartitionSpec("core"),), check_rep=False),
        donate_argnums=(4,), keep_unused=True)
    return fn, mesh


def kernel(x: np.ndarray, key_pts: np.ndarray, roll: np.ndarray, **_kw) -> np.ndarray:
    import jax
    from jax.sharding import NamedSharding, PartitionSpec

    x = np.ascontiguousarray(np.asarray(x, dtype=np.float32))
    key_pts = np.asarray(key_pts, dtype=np.float32)
    roll = np.asarray(roll, dtype=np.float32)

    w_items, metas, mpks = build_schedule(key_pts, roll)
    nc = _get_module(w_items)
    fn, mesh = make_runner(nc)
    sharding = NamedSharding(mesh, PartitionSpec("core"))

    meta_g = jax.device_put(np.concatenate(metas, axis=0), sharding)
    mpk_g = jax.device_put(np.concatenate(mpks, axis=0), sharding)
    eyes_g = jax.device_put(np.concatenate([_eyes4()] * N_CORES, axis=0), sharding)
    reps_g = jax.device_put(np.ones((N_CORES, 1), np.int32), sharding)
    out_g = jax.device_put(x.reshape(N * GROUPS, CHS, NCHUNK, 512), sharding)

    res = fn(meta_g, mpk_g, eyes_g, reps_g, out_g)[0]
    return np.asarray(res).reshape(N, C, H, W)


# revision 20
# speedup vs baseline: 1.0788x; 1.0189x over previous
"""DropPart masking kernel for Trainium2 (8 NeuronCores, data-parallel over batch).

Problem: x (64, 256, 96, 32) f32. Per sample n and channel-group g (8 groups x
32 channels), a keypoint defines a keep-box; if roll[n,g] < 0.5 the group's
channels are zeroed outside the box (box <= 16x16 in the 96x32 image), else the
group passes through unchanged.

The op is pure data movement plus a 0/1 mask multiply: every (n,g) "slot"
([32ch, 96, 32] = 384KB) is either identity or x*boxmask. This kernel runs
IN-PLACE on the device: the full x tensor is donated as the buffer backing the
NEFF's output (XLA donation aliases the param to the custom-call result -- the
same mechanism bass2jax.run_bass_via_pjrt relies on for its zero-filled output
buffers), so identity slots need ZERO HBM traffic and only masked slots are
touched: one 384KB slab read + one 384KB slab write each. At ~50% masked slots
that is ~32MB/core round-trip vs 50MB for the out-of-place streaming kernel,
and the f32 output is bit-exact (rel err 0.0).

Which slots are masked is per-core data, so a single SPMD program executes a
metadata-driven schedule: per work position an i32 input supplies (slot s,
flag f); the slab load/store DMAs take their DRAM offsets from engine
registers (values_load) and padding positions are skipped via cond=f
(skip_entire_dma). dep_tracking_offset on each dynamic AP is pinned to a
static per-position fake region so the Tile scheduler sees disjoint slots
instead of serializing every DMA against the whole output tensor. The mask
multiply uses the eyes-matmul expansion (PE, [4,128] one-hot weights applied
to 4 packed bf16 mask rows) + DVE tensor_mul; masked-slot masks are 1 inside
the keep-box and 0 outside, so the product equals the reference's x*mask
bit-for-bit, and all-ones lanes pass identity data through unchanged. Slab
DMAs rotate across three queues (SP + ACT HWDGE, gpsimd SWDGE) -- the
measured bottleneck is per-dynamic-DMA queue overhead (~2-3us each), not
bandwidth, so the third queue and the one-load-one-store-per-slot structure
are what set the 129us/iter (vs 154us baseline full-stream) timing.

The program is input-independent (one compile per work-list capacity, cached);
the For_i(nreps) wrapper supports in-NEFF repetition for timing. The body is
idempotent (mask o mask = mask), so repeated in-place application is safe.
"""

import numpy as np
import ml_dtypes

import bass_rust
import concourse.bass as bass
import concourse.bacc as bacc
import concourse.tile as tile
from concourse import mybir

N, C, H, W = 64, 256, 96, 32
GROUPS = 8
P_DROP = 0.5
HW = H * W              # 3072
CHS = C // GROUPS       # 32
N_CORES = 8
NPC = N // N_CORES      # samples per core = 8
SLOTS = NPC * GROUPS    # (sample, group) slots per core = 64
NCHUNK = 6              # 512-element chunks per image (96*32 / 512)

_F32 = mybir.dt.float32
_BF16 = mybir.dt.bfloat16
_I32 = mybir.dt.int32

_SP = mybir.EngineType.SP
_ACT = mybir.EngineType.Activation
_POOL = mybir.EngineType.Pool


SLAB = CHS * HW  # elements per slot slab = 98304


def _win_ap(o4, s, c, p):
    """[32, 2, 512] window AP at slab s, chunks [c, c+2): widen the
    [32, 512] single-chunk slice with an extra (stride=512, size=2) dim.

    dep_tracking_offset is pinned to a static per-work-position fake slab so
    the Tile dependency tracker sees disjoint regions per slot (the dynamic
    offsets would otherwise serialize every DMA against the whole tensor).
    Within a position, the window load / zero store / window store share the
    fake region, preserving their required ordering; distinct positions touch
    provably disjoint real slabs, so dropping those edges is sound."""
    a = o4[s, :, c, :].copy()
    a.ap = bass_rust.VecI64Pair([(HW, CHS), (512, 2), (1, 512)])
    a.dep_tracking_offset = p * SLAB
    return a


def _slab_ap(o4, s, p):
    """[32, 6, 512] full-slab AP at slab s, same fake dep region as _win_ap."""
    a = o4[s].copy()
    a.dep_tracking_offset = p * SLAB
    return a


def _nslab_ap(o4, s, p, npart):
    """[npart*32, 6, 512] AP over npart consecutive slabs starting at slab s.
    Fake dep region: 2 slabs per work position (positions never overlap)."""
    a = o4[s].copy()
    a.ap = bass_rust.VecI64Pair([(HW, npart * CHS), (512, NCHUNK), (1, 512)])
    a.dep_tracking_offset = p * 2 * SLAB
    return a


def _build_module(w_key, xbufs: int = 6, mulw: int = 1024):
    """One SPMD module processing w_items*4 slot positions per iteration.

    Per slot position p (metadata: slot s, active flag f), all offsets coming
    from engine registers so one compiled program serves every core's
    schedule:
      L: DMA slab s ([32, 6, 512] f32, 384KB) into the item's [128, 3072]
         SBUF tile (cond=f; inactive positions skip via skip_entire_dma).
      mask multiply: PE expands the 4 packed mask rows (one per 32-channel
         lane) to 128 partitions via the eyes matmul; DVE multiplies the
         whole tile. Masked-slot masks are 1 inside the keep-box and 0
         outside, so the product is exactly the reference's x*mask.
      S: DMA the slab back (cond=f). S chains after L through the SBUF
         dependency (mul), so queues are free; cross-iteration window races
         in the timing loop are benign because masking is idempotent.
    L and S rotate across the SP / ACT HWDGE queues and the gpsimd SWDGE
    queue to spread descriptor-processing overhead and store bandwidth.
    """
    w2, w1 = w_key
    npos = w2 * 2 + w1 * 4
    w_items = w2 + w1
    nc = bacc.Bacc("TRN2", target_bir_lowering=False, debug=False)

    o4 = nc.dram_tensor("out", [SLOTS, CHS, NCHUNK, 512], _F32, kind="ExternalOutput").ap()
    meta_d = nc.dram_tensor("meta", [1, npos * 4], _I32, kind="ExternalInput").ap()
    mpk_d = nc.dram_tensor("mpk", [4, w_items * HW], _BF16, kind="ExternalInput").ap()
    eyes_d = nc.dram_tensor("eyes", [4, 128], _BF16, kind="ExternalInput").ap()
    reps_d = nc.dram_tensor("nreps", [1, 1], _I32, kind="ExternalInput").ap()

    queues = [(nc.sync, _SP), (nc.scalar, _ACT), (nc.gpsimd, _POOL)]
    nbank = mulw // 512

    with tile.TileContext(nc) as tc:
        with (
            tc.tile_pool(name="consts", bufs=1) as consts,
            tc.tile_pool(name="xpool", bufs=xbufs) as xpool,
            tc.tile_pool(name="psum", bufs=8 // nbank, space="PSUM") as psum,
        ):
            mt = consts.tile([1, npos * 4], _I32)
            nc.sync.dma_start(mt[:], meta_d[:])
            mpk = consts.tile([4, w_items * HW], _BF16)
            nc.sync.dma_start(mpk[:], mpk_d[:])
            eyes = consts.tile([4, 128], _BF16)
            nc.sync.dma_start(eyes[:], eyes_d[:])
            rtile = consts.tile([1, 1], _I32)
            nc.sync.dma_start(rtile[:], reps_d[:])

            dmac = [0]  # global DMA counter for queue rotation

            def unit_io(it, units, npart):
                """Loads for one item: `units` dynamic units of `npart*32`
                partitions each. Returns regs for the store pass."""
                xt = xpool.tile([128, HW], _F32)
                regs = []
                for u in range(units):
                    p = it_pos(it, u, units)
                    qL, engL = queues[dmac[0] % 3]
                    qS, engS = queues[(dmac[0] + 1) % 3]
                    dmac[0] += 1
                    engs = [engL] if engS == engL else [engL, engS]
                    s = nc.values_load(mt[0:1, 4 * p : 4 * p + 1], engines=engs,
                                       min_val=0, max_val=SLOTS - npart,
                                       skip_runtime_bounds_check=True)
                    f = nc.values_load(mt[0:1, 4 * p + 2 : 4 * p + 3], engines=engs,
                                       min_val=0, max_val=1,
                                       skip_runtime_bounds_check=True)
                    regs.append((qS, s, f))
                    rows = 32 * npart
                    xk3 = xt[rows * u : rows * (u + 1), :].rearrange(
                        "p (a b) -> p a b", b=512)
                    qL.dma_start(xk3, _nslab_ap(o4, s, p, npart), cond=f, cond_hint=True)
                return xt, regs

            def mask_mul(xt, mcol):
                for j in range(HW // mulw):
                    pt = psum.tile([128, mulw], _F32)
                    for b in range(nbank):
                        col = mcol * HW + j * mulw + b * 512
                        nc.tensor.matmul(pt[:, 512 * b : 512 * b + 512], eyes[:],
                                         mpk[:, col : col + 512],
                                         start=True, stop=True)
                    nc.vector.tensor_mul(xt[:, j * mulw : (j + 1) * mulw],
                                         xt[:, j * mulw : (j + 1) * mulw], pt[:])

            def unit_store(xt, regs, it, units, npart):
                for u in range(units):
                    p = it_pos(it, u, units)
                    qS, s, f = regs[u]
                    rows = 32 * npart
                    xk3 = xt[rows * u : rows * (u + 1), :].rearrange(
                        "p (a b) -> p a b", b=512)
                    qS.dma_start(_nslab_ap(o4, s, p, npart), xk3, cond=f, cond_hint=True)

            def it_pos(it, u, units):
                # pair-items occupy meta positions [0, w2*2); single-items after
                return it * 2 + u if units == 2 else w2 * 2 + (it - w2) * 4 + u

            with tc.For_i(0, nc.values_load(rtile[0:1, 0:1]), 1):
                for it in range(w2 + w1):
                    units, npart = (2, 2) if it < w2 else (4, 1)
                    xt, regs = unit_io(it, units, npart)
                    mask_mul(xt, it)
                    unit_store(xt, regs, it, units, npart)

    nc.compile()
    return nc


_MODULES: dict = {}


def _get_module(w_items: int):
    if w_items not in _MODULES:
        _MODULES[w_items] = _build_module(w_items)
    return _MODULES[w_items]


def _host_masks(key_pts: np.ndarray, roll: np.ndarray) -> np.ndarray:
    """Per-(n,g) masks [N, GROUPS, H*W] in {0,1} f32, math exactly as reference."""
    s = int(0.25 * W)
    kx = (key_pts[:, :GROUPS, 0] * np.float32(W)).astype(np.float32)
    ky = (key_pts[:, :GROUPS, 1] * np.float32(H)).astype(np.float32)
    cond = (roll[:, :GROUPS] < np.float32(P_DROP)) & (kx >= 0) & (ky >= 0)

    bx = np.floor(np.maximum(kx - s, np.float32(0.0)))
    ex = np.floor(np.minimum(kx + s, np.float32(W)))
    by = np.floor(np.maximum(ky - s, np.float32(0.0)))
    ey = np.floor(np.minimum(ky + s, np.float32(H)))

    xs = np.arange(W, dtype=np.float32)
    ys = np.arange(H, dtype=np.float32)
    inx = (xs[None, None, :] >= bx[:, :, None]) & (xs[None, None, :] < ex[:, :, None])
    iny = (ys[None, None, :] >= by[:, :, None]) & (ys[None, None, :] < ey[:, :, None])
    box = iny[:, :, :, None] & inx[:, :, None, :]  # [N, G, H, W] bool

    mask = np.where(cond[:, :, None, None], box, True)
    return mask.reshape(N, GROUPS, HW).astype(np.float32)


def build_schedule(key_pts: np.ndarray, roll: np.ndarray):
    """Host schedule: per-core packed work lists from the mask table.

    Returns (w_items, metas [8][1, nslots*4] i32, mpks [8][nslots, 1024] bf16).
    A slot is active iff its mask differs from all-ones; its window chunk c is
    chosen from the mask's nonzero rows so that chunks [c, c+2) cover them.
    """
    masks = _host_masks(key_pts, roll)  # [N, G, HW] f32 0/1
    m_core = masks.reshape(N_CORES, SLOTS, H, W)
    # Exact cover: adjacent masked slots (s, s+1) merge into one 2-slab unit
    # (one 768KB L + one S); isolated masked slots stay 1-slab units.
    pairs_w, singles_w = [], []
    for cidx in range(N_CORES):
        masked = [sl for sl in range(SLOTS) if m_core[cidx, sl].min() < 1.0]
        pairs, singles = [], []
        i = 0
        while i < len(masked):
            if i + 1 < len(masked) and masked[i + 1] == masked[i] + 1:
                pairs.append(masked[i]); i += 2
            else:
                singles.append(masked[i]); i += 1
        pairs_w.append(pairs); singles_w.append(singles)

    w2 = min(16, max(1, max(-(-len(p) // 2) for p in pairs_w)))   # pair-items (2 units)
    w1 = min(16, max(1, max(-(-len(s) // 4) for s in singles_w))) # single-items (4 units)
    npos = w2 * 2 + w1 * 4

    metas, mpks = [], []
    flat = masks.reshape(N_CORES, SLOTS, HW)
    for cidx in range(N_CORES):
        meta = np.zeros((1, npos * 4), dtype=np.int32)
        mpk = np.zeros((4, (w2 + w1) * HW), dtype=np.float32)
        for p, sl in enumerate(pairs_w[cidx]):       # pair positions 0..w2*2
            meta[0, 4 * p : 4 * p + 3] = (sl, 0, 1)
            it, u = divmod(p, 2)
            for j in range(2):
                mpk[2 * u + j, it * HW : (it + 1) * HW] = flat[cidx, sl + j]
        for q, sl in enumerate(singles_w[cidx]):     # single positions after pairs
            p = w2 * 2 + q
            meta[0, 4 * p : 4 * p + 3] = (sl, 0, 1)
            it, k = divmod(q, 4)
            mpk[k, (w2 + it) * HW : (w2 + it + 1) * HW] = flat[cidx, sl]
        metas.append(meta)
        mpks.append(mpk.astype(ml_dtypes.bfloat16))
    return (w2, w1), metas, mpks


def _eyes4() -> np.ndarray:
    e = np.zeros((4, 128), dtype=np.float32)
    for k in range(4):
        e[k, 32 * k : 32 * k + 32] = 1.0
    return e.astype(ml_dtypes.bfloat16)


def make_runner(nc):
    """jit'd shard_map runner over 8 cores with the 'out' buffer donated.

    Returns fn(meta_g, mpk_g, eyes_g, nreps_g, out_g) -> out_g ([512,32,6,512]
    f32 jax array). out_g is consumed (donated); chain calls by passing the
    previous result.
    """
    import jax
    from jax.sharding import Mesh, PartitionSpec
    from jax.experimental.shard_map import shard_map
    from concourse.bass2jax import (
        _bass_exec_p,
        install_neuronx_cc_hook,
        partition_id_tensor,
    )

    install_neuronx_cc_hook()
    partition_name = nc.partition_id_tensor.name if nc.partition_id_tensor else None

    in_names, out_names, out_avals = [], [], []
    for alloc in nc.m.functions[0].allocations:
        if not isinstance(alloc, mybir.MemoryLocationSet):
            continue
        name = alloc.memorylocations[0].name
        if alloc.kind == "ExternalInput":
            if name != partition_name:
                in_names.append(name)
        elif alloc.kind == "ExternalOutput":
            out_names.append(name)
            out_avals.append(jax.core.ShapedArray(tuple(alloc.tensor_shape),
                                                  mybir.dt.np(alloc.dtype)))
    assert out_names == ["out"]
    order = ["meta", "mpk", "eyes", "nreps"]
    assert sorted(in_names) == sorted(order), in_names
    perm = [order.index(n) for n in in_names]
    all_names = tuple(in_names) + ("out",)
    if partition_name is not None:
        all_names = all_names + (partition_name,)

    def _body(*args):
        operands = [args[perm[i]] for i in range(4)] + [args[4]]
        if partition_name is not None:
            operands.append(partition_id_tensor())
        (res,) = _bass_exec_p.bind(
            *operands,
            out_avals=tuple(out_avals),
            in_names=all_names,
            out_names=("out",),
            lowering_input_output_aliases=(),
            sim_require_finite=False, sim_require_nnan=False, nc=nc)
        return (res,)

    mesh = Mesh(np.asarray(jax.devices()[:N_CORES]), ("core",))
    specs = (PartitionSpec("core"),) * 5
    fn = jax.jit(
        shard_map(_body, mesh=mesh, in_specs=specs,
                  out_specs=(PartitionSpec("core"),), check_rep=False),
        donate_argnums=(4,), keep_unused=True)
    return fn, mesh


def kernel(x: np.ndarray, key_pts: np.ndarray, roll: np.ndarray, **_kw) -> np.ndarray:
    import jax
    from jax.sharding import NamedSharding, PartitionSpec

    x = np.ascontiguousarray(np.asarray(x, dtype=np.float32))
    key_pts = np.asarray(key_pts, dtype=np.float32)
    roll = np.asarray(roll, dtype=np.float32)

    w_items, metas, mpks = build_schedule(key_pts, roll)
    nc = _get_module(w_items)
    fn, mesh = make_runner(nc)
    sharding = NamedSharding(mesh, PartitionSpec("core"))

    meta_g = jax.device_put(np.concatenate(metas, axis=0), sharding)
    mpk_g = jax.device_put(np.concatenate(mpks, axis=0), sharding)
    eyes_g = jax.device_put(np.concatenate([_eyes4()] * N_CORES, axis=0), sharding)
    reps_g = jax.device_put(np.ones((N_CORES, 1), np.int32), sharding)
    out_g = jax.device_put(x.reshape(N * GROUPS, CHS, NCHUNK, 512), sharding)

    res = fn(meta_g, mpk_g, eyes_g, reps_g, out_g)[0]
    return np.asarray(res).reshape(N, C, H, W)
